# revision 32
# baseline (speedup 1.0000x reference)
"""Sharded causal-attention kernel for 8 trn2 NeuronCores.

DP over batch (2) x TP over head groups (4 heads/core). Each core: qkv projection
(its heads) + RoPE + causal SDPA (scores kept transposed; softmax denominator via a
ones-column in the PV matmul) + its 256-row slice of the o_proj contraction, returning
a transposed partial [HID, S]; the host sums 4 partials per batch. bf16 matmuls,
fp32 PSUM accumulation.

Schedule: the attention inner loop is exp(ACT)-gated, so projection / v / o_proj
work is emitted as small "filler" units threaded between the QK and PV matmuls of
each key tile, keeping the PE FIFO saturated while ACT grinds exps. Input DMAs are
kk-granular across four engine queues so the first matmul can start ~3us in; output
DMAs rotate across queues so the store drains while compute proceeds.
"""

import sys

sys.path.insert(0, "/opt/trn_rl_repo")

from collections import deque
from contextlib import ExitStack

import numpy as np
import ml_dtypes

import concourse.bass as bass
import concourse.mybir as mybir
import concourse.tile as tile
from concourse import bacc

FP = mybir.dt.float32
BF = mybir.dt.bfloat16
EXP = mybir.ActivationFunctionType.Exp

B, S, HID = 2, 2048, 1024
H, D = 16, 64
QC = 512
KT = 128
NQC = S // QC
NKT = S // KT
KHID = HID // 128


def build_program(debug_outputs=False):
    nc = bacc.Bacc("TRN2", target_bir_lowering=False, debug=False, num_devices=8, num_swdge_queues=2)

    hsT = nc.dram_tensor("hsT", [128, NQC * KHID * QC], BF, kind="ExternalInput").ap()
    wqkT = nc.dram_tensor("wqkT", [128, KHID * 512], BF, kind="ExternalInput").ap()
    wvT = nc.dram_tensor("wvT", [128, KHID * 256], BF, kind="ExternalInput").ap()
    woT = nc.dram_tensor("woT", [128, 2 * HID], BF, kind="ExternalInput").ap()
    cos2T = nc.dram_tensor("cos2T", [64, S], BF, kind="ExternalInput").ap()
    ssin2T = nc.dram_tensor("ssin2T", [64, S], BF, kind="ExternalInput").ap()
    maskD = nc.dram_tensor("maskD", [128, 256], BF, kind="ExternalInput").ap()
    pmat = nc.dram_tensor("pmat", [128, 128], BF, kind="ExternalInput").ap()
    outT = nc.dram_tensor("outT", [128, NQC * 8 * QC], BF, kind="ExternalOutput").ap()

    with tile.TileContext(nc) as tc:
        build_tile_program(tc, hsT, wqkT, wvT, woT, cos2T, ssin2T, maskD, pmat, outT)
    nc.compile()
    return nc


def build_tile_program(tc, hsT, wqkT, wvT, woT, cos2T, ssin2T, maskD, pmat, outT):
    nc = tc.nc
    with ExitStack() as ctx:
        const = ctx.enter_context(tc.tile_pool(name="const", bufs=1))
        persist = ctx.enter_context(tc.tile_pool(name="persist", bufs=1))
        work = ctx.enter_context(tc.tile_pool(name="work", bufs=5))
        posbp = ctx.enter_context(tc.tile_pool(name="posbp", bufs=6))
        expp = ctx.enter_context(tc.tile_pool(name="expp", bufs=8))
        small = ctx.enter_context(tc.tile_pool(name="small", bufs=4))
        ps_main = ctx.enter_context(tc.tile_pool(name="ps_main", bufs=2, space="PSUM"))
        ps_sc = ctx.enter_context(tc.tile_pool(name="ps_sc", bufs=2, space="PSUM"))
        ps_po = ctx.enter_context(tc.tile_pool(name="ps_po", bufs=2, space="PSUM"))

        # ---- SBUF destination tiles ----
        wqk_sb = const.tile([128, KHID, 512], BF, name="wqk_sb")
        hs_sb = const.tile([128, NQC, KHID, QC], BF, name="hs_sb")
        cos_sb = const.tile([128, S], BF, name="cos_sb")
        ssin_sb = const.tile([128, S], BF, name="ssin_sb")
        tri_sb = const.tile([128, 2, 128], BF, name="tri_sb")
        pmat_sb = const.tile([128, 128], BF, name="pmat_sb")
        wv_sb = const.tile([128, KHID, 256], BF, name="wv_sb")
        wo_sb = const.tile([128, 2, HID], BF, name="wo_sb")

        # ---- input DMAs: kk-granular, spread over four engine queues, in
        # priority order (wqk + hs chunk0 first so the first matmul can start
        # ~3us in; later chunks stream behind the compute). ----
        def hs_piece(t, k2):  # kk pair [2k2, 2k2+2) of chunk t
            return (
                hs_sb[:, t, 2 * k2:2 * k2 + 2, :],
                hsT[:, t * KHID * QC + 2 * k2 * QC: t * KHID * QC + (2 * k2 + 2) * QC]
                .rearrange("p (k s) -> p k s", k=2),
            )

        def wqk_piece(k2):
            return (
                wqk_sb[:, 2 * k2:2 * k2 + 2, :],
                wqkT[:, 2 * k2 * 512:(2 * k2 + 2) * 512].rearrange("p (k m) -> p k m", k=2),
            )

        # warm the ACT table (exp set loads ~2.7us) while DMAs stream
        warm = const.tile([1, 16], FP, name="warm")
        nc.scalar.activation(warm[:], warm[:], EXP, scale=0.0)

        def hs_half(t, h):  # kk half [4h, 4h+4) of chunk t
            return (
                hs_sb[:, t, 4 * h:4 * h + 4, :],
                hsT[:, t * KHID * QC + 4 * h * QC: t * KHID * QC + (4 * h + 4) * QC]
                .rearrange("p (k s) -> p k s", k=4),
            )

        def wqk_one(k):
            return (wqk_sb[:, k, :], wqkT[:, k * 512:(k + 1) * 512])

        def hs_one(t, k):
            return (hs_sb[:, t, k, :],
                    hsT[:, t * KHID * QC + k * QC: t * KHID * QC + (k + 1) * QC])

        qS, qG, qC = nc.sync, nc.gpsimd, nc.scalar
        plan = [
            (qS, wqk_one(0)), (qG, hs_one(0, 0)),
            (qS, wqk_one(1)), (qG, hs_one(0, 1)),
            (qC, hs_piece(0, 1)),
            (qS, wqk_piece(1)), (qG, hs_piece(0, 2)), (qC, hs_piece(0, 3)),
            (qS, wqk_piece(2)), (qG, hs_piece(1, 0)), (qC, (pmat_sb[:], pmat[:])),
            (qS, wqk_piece(3)), (qG, hs_piece(1, 1)),
            (qC, (cos_sb[0:64, :], cos2T[:])),
            (qC, (cos_sb[64:128, :], cos2T[:])),
            (qS, hs_piece(1, 2)), (qG, hs_piece(1, 3)),
            (qC, (ssin_sb[0:64, :], ssin2T[:])),
            (qC, (ssin_sb[64:128, :], ssin2T[:])),
            (qS, (tri_sb[:], maskD.rearrange("p (r c) -> p r c", r=2))),
            (qG, (wv_sb[:, 0:4, :], wvT[:, 0:1024].rearrange("p (k m) -> p k m", k=4))),
            (qS, (wv_sb[:, 4:8, :], wvT[:, 1024:2048].rearrange("p (k m) -> p k m", k=4))),
            (qG, hs_half(2, 0)), (qS, hs_half(2, 1)),
            (qC, (wo_sb[:], woT.rearrange("p (k m) -> p k m", k=2))),
            (qG, hs_half(3, 0)), (qS, hs_half(3, 1)),
        ]
        for eng, (dst, src) in plan:
            eng.dma_start(dst, src)

        # HAM warm-up: ~24 back-to-back matmuls over the first-landed pieces.
        # Re-warming to 2.4 GHz needs a sustained burst of PE activity; the
        # DMA-bound start is otherwise too choppy to trigger it, leaving the
        # PE at 1.2 GHz well into the attention phase. Result is discarded.
        wps = ps_main.tile([128, QC], FP, name="warm_ps", tag="ps")
        for _ in range(24):
            nc.tensor.matmul(wps[:], wqk_sb[:, 0, 0:128], hs_sb[:, 0, 0, :],
                             start=True, stop=True)

        qkT = persist.tile([128, 4, S], BF, name="qkT")
        v_sb = persist.tile([128, NKT, 4 * 65], BF, name="v_sb2")
        att_sb = persist.tile([128, 2, S], BF, name="att_sb2")
        nc.vector.memset(
            v_sb.rearrange("p t (h c) -> p t h c", c=65)[:, :, :, 64:65], 1.0
        )

        # ---- work units (each emits a bounded slug of PE work + its
        # elementwise tail on DVE/pool, or ACT in the DMA-bound pre-phase) ----

        def proj_a(rb, t):
            # first half of the qk projection for (rb, t): 4 contraction steps
            ps = ps_main.tile([128, QC], FP, name="ps_qk", tag="ps")
            for kk in range(4):
                nc.tensor.matmul(
                    ps[:], wqk_sb[:, kk, rb * 128:(rb + 1) * 128],
                    hs_sb[:, t, kk, :], start=(kk == 0), stop=False,
                )
            return ps

        def proj_b(rb, t, ps, act=False):
            # second half + RoPE
            csl = slice(t * QC, (t + 1) * QC)
            for kk in range(4, KHID):
                nc.tensor.matmul(
                    ps[:], wqk_sb[:, kk, rb * 128:(rb + 1) * 128],
                    hs_sb[:, t, kk, :], start=False, stop=(kk == KHID - 1),
                )
            x = work.tile([128, QC], BF, name="x_rope", tag="xrope")
            if act:
                nc.scalar.copy(x[:], ps[:])
            else:
                nc.vector.tensor_copy(x[:], ps[:])
            xs_ps = ps_main.tile([128, QC], FP, name="xs_ps", tag="ps")
            nc.tensor.matmul(xs_ps[:], pmat_sb[:], x[:], start=True, stop=True)
            t1 = work.tile([128, QC], BF, name="t1_rope", tag="t1rope")
            t2 = work.tile([128, QC], BF, name="t2_rope", tag="t2rope")
            nc.vector.tensor_mul(t1[:], x[:], cos_sb[:, csl])
            nc.vector.tensor_mul(t2[:], xs_ps[:], ssin_sb[:, csl])
            nc.vector.tensor_add(qkT[:, rb, csl], t1[:], t2[:])

        def v_unit(tt, act=False):
            psv = ps_main.tile([128, 256], FP, name="ps_v", tag="ps")
            for kk in range(KHID):
                nc.tensor.matmul(
                    psv[:],
                    hs_sb[:, tt // 4, kk, (tt % 4) * 128:(tt % 4 + 1) * 128],
                    wv_sb[:, kk, :], start=(kk == 0), stop=(kk == KHID - 1),
                )
            dst = v_sb[:, tt, :].rearrange("p (h c) -> p h c", c=65)[:, :, 0:64]
            src = psv[:].rearrange("p (h c) -> p h c", c=64)
            if act:
                nc.scalar.copy(dst, src)
            else:
                nc.vector.tensor_copy(dst, src)

        out_q = [nc.sync, nc.gpsimd]

        def oproj_ot(qi, half, oi, ow, act=False):
            qsl = slice(qi * QC, (qi + 1) * QC)
            ot = half * 4 + oi
            pw = ps_main.tile([128, QC], FP, name="pw", tag="ps")
            for p in range(2):
                nc.tensor.matmul(
                    pw[:], wo_sb[:, p, ot * 128:(ot + 1) * 128],
                    att_sb[:, p, qsl], start=(p == 0), stop=(p == 1),
                )
            if act or oi % 2 == 1:
                nc.scalar.copy(ow[:, oi, :], pw[:])
            else:
                nc.vector.tensor_copy(ow[:, oi, :], pw[:])
            if oi % 2 == 1:
                off = (qi * 2 + half) * 4 * QC + (oi - 1) * QC
                eng = out_q[(qi * 2 + half + oi) % 2]
                eng.dma_start(
                    outT[:, off:off + 2 * QC].rearrange("p (o s) -> p o s", o=2),
                    ow[:, oi - 1:oi + 1, :],
                )

        # filler machinery: (name, cost_ns, closure). Emission order == PE FIFO
        # order, so any attention matmul must be emitted after every unit it
        # reads from; require() flushes the deque up to a named unit. pump()
        # spends an accumulated credit so filler emission is paced smoothly
        # (bursts in the PE FIFO starve ACT, the attention rate-setter).
        fillers = deque()
        emitted = set()
        credit = [0]

        def pump(budget):
            while budget > 0 and fillers:
                name, cost, fn = fillers.popleft()
                fn()
                emitted.add(name)
                budget -= cost

        def require(*names):
            while any(n not in emitted for n in names) and fillers:
                name, cost, fn = fillers.popleft()
                fn()
                emitted.add(name)

        def push_proj(rb, t):
            st = {}
            fillers.append((f"pa{rb}_{t}", 900, lambda: st.__setitem__("ps", proj_a(rb, t))))
            fillers.append((f"p{rb}_{t}", 1100, lambda: proj_b(rb, t, st["ps"])))

        def push_v(t):
            for tt in range(4 * t, 4 * t + 4):
                fillers.append((f"v{tt}", 1000, lambda tt=tt: v_unit(tt)))

        def push_oproj(qi, act_all=False):
            st = {}
            for half in range(2):
                def mk(half):
                    def alloc():
                        st[half] = work.tile([128, 4, QC], BF, name="ow", tag="ow")
                    return alloc
                alloc = mk(half)
                for oi in range(4):
                    def u(half=half, oi=oi, alloc=alloc):
                        if oi == 0:
                            alloc()
                        oproj_ot(qi, half, oi, st[half], act=act_all)
                    fillers.append((f"o{qi}_{half}_{oi}", 650, u))

        # ---- attention ----

        def attention_unit(pair, qi):
            """Emits scores^T -> exp -> PV for heads (2*pair, 2*pair+1) of query
            chunk qi, with filler units threaded between QK(ki) and PV(ki-1) so
            the PE keeps working while ACT runs the exp. Returns (po0, po1)."""
            qsl = slice(qi * QC, (qi + 1) * QC)
            nki = 4 * qi + 4
            po0 = ps_po.tile([65, QC], FP, name="po0", tag="po")
            po1 = ps_po.tile([65, QC], FP, name="po1", tag="po")
            pend = None  # (ki, e, lo) awaiting PV

            def emit_pv(ki, e, lo):
                h0 = 2 * pair
                h1 = 2 * pair + 1
                nc.tensor.matmul(
                    po0[:, lo:QC], v_sb[:, ki, h0 * 65:(h0 + 1) * 65], e[:, 0, lo:QC],
                    start=(ki == 0), stop=(ki == nki - 1),
                )
                nc.tensor.matmul(
                    po1[:, lo:QC], v_sb[:, ki, h1 * 65:(h1 + 1) * 65], e[:, 1, lo:QC],
                    start=(ki == 0), stop=(ki == nki - 1),
                )

            require(f"p{pair}_{qi}", f"p{2 + pair}_0")
            for ki in range(nki):
                ksl = slice(ki * KT, (ki + 1) * KT)
                j = ki - 4 * qi
                lo = 0 if j < 0 else 128 * j  # first live q column in this tile
                qslh = slice(qi * QC + lo, (qi + 1) * QC)
                if ki % 4 == 0:
                    require(f"p{2 + pair}_{ki // 4}")
                psc = ps_sc.tile([128, 2, QC], FP, name="psc", tag="sc")
                nc.tensor.matmul(
                    psc[:, 0, lo:QC], qkT[0:64, 2 + pair, ksl], qkT[0:64, pair, qslh],
                    start=True, stop=True,
                )
                nc.tensor.matmul(
                    psc[:, 1, lo:QC], qkT[64:128, 2 + pair, ksl], qkT[64:128, pair, qslh],
                    start=True, stop=True,
                )
                e = expp.tile([128, 2, QC], BF, name="e", tag="exp")
                if lo == 0:
                    # contiguous flat view: one run on ACT instead of two rows
                    nc.scalar.activation(
                        e.rearrange("p a b -> p (a b)")[:, :],
                        psc.rearrange("p a b -> p (a b)")[:, :], EXP, scale=0.125,
                    )
                else:
                    nc.scalar.activation(
                        e[:, :, lo:QC], psc[:, :, lo:QC], EXP, scale=0.125
                    )
                if j >= 0:
                    nc.vector.tensor_mul(
                        e[:, :, lo:lo + 128], e[:, :, lo:lo + 128], tri_sb[:]
                    )
                if pend is not None:
                    if j <= 0:
                        pump(480)
                    elif j == 1:
                        pump(200)
                    require(f"v{pend[0]}")
                    emit_pv(*pend)
                pend = (ki, e, lo)
            pump(150)
            require(f"v{pend[0]}")
            emit_pv(*pend)
            # free the po banks promptly: numerators+denominator to SBUF via
            # ACT, which is idle exactly at these unit boundaries (no exps);
            # the normalization then runs off the PSUM-release path
            po_sb0 = posbp.tile([65, QC], FP, name="po_sb0", tag="posb")
            po_sb1 = posbp.tile([65, QC], FP, name="po_sb1", tag="posb")
            nc.scalar.copy(po_sb0[:], po0[:])
            nc.scalar.copy(po_sb1[:], po1[:])
            return po_sb0, po_sb1

        def division(pair, qi, po_sb0, po_sb1):
            """att = po / l from the SBUF copies, off the PSUM-release path."""
            qsl = slice(qi * QC, (qi + 1) * QC)
            for sub, posb in enumerate([po_sb0, po_sb1]):
                l_sb = small.tile([1, QC], FP, name="l_sb", tag="lsb", bufs=4)
                nc.vector.tensor_copy(l_sb[:], posb[64:65, :])
                rl = small.tile([1, QC], FP, name="rl", tag="rl", bufs=4)
                nc.vector.reciprocal_approx_fast(out=rl[:], in_=l_sb[:])
                rb_ = small.tile([64, QC], FP, name="rb_", tag="rbb", bufs=4)
                nc.gpsimd.partition_broadcast(rb_[:], rl[:])
                nc.vector.tensor_mul(
                    att_sb[sub * 64:(sub + 1) * 64, pair, qsl], posb[0:64, :], rb_[:]
                )

        # ---- emission schedule ----
        # pre-phase (DMA-bound): just the two units the first attention QKs
        # need, with PSUM->SBUF copies on the otherwise-idle ACT engine;
        # everything else becomes paced filler, flushed on demand by the
        # fine-grained require() calls inside attention_unit.
        for rb, t in [(2, 0), (0, 1)]:
            ps = proj_a(rb, t)
            proj_b(rb, t, ps, act=True)
            emitted.add(f"p{rb}_{t}")
        for tt in range(4):
            v_unit(tt, act=True)
            emitted.add(f"v{tt}")

        push_proj(2, 1)
        for tt in range(4, 8):
            fillers.append((f"v{tt}", 1000, lambda tt=tt: v_unit(tt)))
        push_proj(3, 0); push_proj(1, 1); push_proj(3, 1)        # att(1, p1)
        push_proj(0, 0); push_proj(1, 0)                         # att(0)
        push_proj(0, 2); push_proj(2, 2); push_v(2)              # att(2, p0)
        push_proj(1, 2); push_proj(3, 2)                         # att(2, p1)
        push_proj(0, 3); push_proj(2, 3); push_v(3)              # att(3, p0)
        push_proj(1, 3); push_proj(3, 3)                         # att(3, p1)

        order = [1, 2, 3, 0]
        for qi in order:
            for pair in range(2):
                pos = attention_unit(pair, qi)
                division(pair, qi, *pos)
            if qi != order[-1]:
                push_oproj(qi)

        # drain leftovers, then the final o_proj chunk with copies alternating
        # between the now-idle ACT and DVE, per-ot DMAs over three queues
        require(*[n for n, _, _ in list(fillers)])
        ql = order[-1]
        tail_q = [nc.sync, nc.gpsimd, nc.scalar]
        for half in range(2):
            ow = work.tile([128, 4, QC], BF, name="ow", tag="ow")
            for oi in range(4):
                qsl = slice(ql * QC, (ql + 1) * QC)
                ot = half * 4 + oi
                pw = ps_main.tile([128, QC], FP, name="pw", tag="ps")
                for p in range(2):
                    nc.tensor.matmul(
                        pw[:], wo_sb[:, p, ot * 128:(ot + 1) * 128],
                        att_sb[:, p, qsl], start=(p == 0), stop=(p == 1),
                    )
                if oi % 2 == 0:
                    nc.scalar.copy(ow[:, oi, :], pw[:])
                else:
                    nc.vector.tensor_copy(ow[:, oi, :], pw[:])
                off = (ql * 2 + half) * 4 * QC + oi * QC
                tail_q[(half * 4 + oi) % 3].dma_start(
                    outT[:, off:off + QC], ow[:, oi, :],
                )


# ---------- host-side shard preparation ----------

def make_core_inputs(hidden_states, cos, sin, w_qkv, w_o):
    """Returns list of 8 in_maps (numpy, bf16 where needed)."""
    bf = ml_dtypes.bfloat16
    hs = np.asarray(hidden_states, np.float32)
    cos = np.asarray(cos, np.float32)
    sin = np.asarray(sin, np.float32)
    w_qkv = np.asarray(w_qkv, np.float32)
    w_o = np.asarray(w_o, np.float32)

    cosT = cos.T
    sinT = sin.T
    cos2T = np.ascontiguousarray(cosT).astype(bf)
    ssin2T = np.ascontiguousarray(sinT).astype(bf)
    # signed rotate-half permutation: out[m] = sign(m) * x[partner(m)]
    # lhsT layout: pmat[k, m] = sign(m) at k = partner(m)
    pmat = np.zeros((128, 128), np.float32)
    for m in range(128):
        d = m % 64
        base = m - d
        if d < 32:
            pmat[base + d + 32, m] = -1.0
        else:
            pmat[base + d - 32, m] = 1.0
    pmat = pmat.astype(bf)

    kp = np.arange(128)[:, None]
    cc = np.arange(128)[None, :]
    tri = (kp <= cc).astype(bf)
    maskD = np.concatenate([tri, tri], axis=1)

    def swz(a):
        # [K*128, M] -> [128, K*M] partition-major (matches SBUF tiles)
        k = a.shape[0] // 128
        return np.ascontiguousarray(
            a.reshape(k, 128, a.shape[1]).transpose(1, 0, 2).reshape(128, -1))

    in_maps = []
    for c in range(8):
        b, g = divmod(c, 4)
        heads = range(4 * g, 4 * g + 4)
        hsT = np.ascontiguousarray(hs[b].T).astype(bf)  # [HID, S]
        # chunk-major swizzle: [128, t, kk, s']
        hs2 = np.ascontiguousarray(
            hsT.reshape(8, 128, 4, 512).transpose(1, 2, 0, 3).reshape(128, -1))
        wq = np.concatenate([w_qkv[h * 64:(h + 1) * 64] for h in heads], 0)
        wk = np.concatenate([w_qkv[HID + h * 64:HID + (h + 1) * 64] for h in heads], 0)
        wv = np.concatenate([w_qkv[2 * HID + h * 64:2 * HID + (h + 1) * 64] for h in heads], 0)
        wqkT = swz(np.ascontiguousarray(np.concatenate([wq, wk], 0).T).astype(bf))
        wvT = swz(np.ascontiguousarray(wv.T).astype(bf))
        woT = swz(np.ascontiguousarray(
            np.concatenate([w_o[:, h * 64:(h + 1) * 64] for h in heads], 1).T
        ).astype(bf))
        in_maps.append({
            "hsT": hs2, "wqkT": wqkT, "wvT": wvT, "woT": woT,
            "cos2T": cos2T, "ssin2T": ssin2T, "maskD": maskD, "pmat": pmat,
        })
    return in_maps


def unswizzle_out(o2):
    # [128, qi*half*oi*512] -> outT [1024, 2048]
    a = o2.reshape(128, NQC, 2, 4, QC)
    return np.ascontiguousarray(
        a.transpose(2, 3, 0, 1, 4).reshape(HID, S))


def unshard(outTs):
    out = np.zeros((B, S, HID), np.float32)
    for c, o2 in enumerate(outTs):
        out[c // 4] += unswizzle_out(o2).T.astype(np.float32)
    return out


# ---------- standalone kernel entry ----------

from concourse.bass_utils import run_bass_kernel_spmd

_CACHED_NC = None


def get_program():
    global _CACHED_NC
    if _CACHED_NC is None:
        _CACHED_NC = build_program()
    return _CACHED_NC


def run(inputs, trace=False):
    nc = get_program()
    in_maps = make_core_inputs(**inputs)
    res = run_bass_kernel_spmd(nc, in_maps, core_ids=list(range(8)), trace=trace)
    out = np.zeros((B, S, HID), np.float32)
    for c, r in enumerate(res.results):
        out[c // 4] += unswizzle_out(r["outT"]).T.astype(np.float32)
    return out, res


def kernel(**inputs):
    out, _ = run(inputs, trace=False)
    return out


# revision 33
# speedup vs baseline: 1.2028x; 1.2028x over previous
"""Sharded causal-attention kernel for 8 trn2 NeuronCores.

DP over batch (2) x TP over head groups (4 heads/core). Each core: qkv projection
(its heads) + RoPE + causal SDPA (scores kept transposed; softmax denominator via a
ones-column in the PV matmul) + its 256-row slice of the o_proj contraction, returning
a transposed partial [HID, S]; the host sums 4 partials per batch. bf16 matmuls,
fp32 PSUM accumulation.

Schedule: the attention inner loop is exp(ACT)-gated, so projection / v / o_proj
work is emitted as small "filler" units threaded between the QK and PV matmuls of
each key tile, keeping the PE FIFO saturated while ACT grinds exps. Input DMAs are
kk-granular across four engine queues so the first matmul can start ~3us in; output
DMAs rotate across queues so the store drains while compute proceeds.
"""

import sys

sys.path.insert(0, "/opt/trn_rl_repo")

from collections import deque
from contextlib import ExitStack

import numpy as np
import ml_dtypes

import concourse.bass as bass
import concourse.mybir as mybir
import concourse.tile as tile
from concourse import bacc

FP = mybir.dt.float32
BF = mybir.dt.bfloat16
EXP = mybir.ActivationFunctionType.Exp

B, S, HID = 2, 2048, 1024
H, D = 16, 64
QC = 512
KT = 128
NQC = S // QC
NKT = S // KT
KHID = HID // 128


def build_program(debug_outputs=False):
    nc = bacc.Bacc("TRN2", target_bir_lowering=False, debug=False, num_devices=8, num_swdge_queues=2)

    hsT = nc.dram_tensor("hsT", [128, NQC * KHID * QC], BF, kind="ExternalInput").ap()
    wqkT = nc.dram_tensor("wqkT", [128, KHID * 512], BF, kind="ExternalInput").ap()
    wvT = nc.dram_tensor("wvT", [128, KHID * 256], BF, kind="ExternalInput").ap()
    woT = nc.dram_tensor("woT", [128, 2 * HID], BF, kind="ExternalInput").ap()
    cos2T = nc.dram_tensor("cos2T", [64, S], BF, kind="ExternalInput").ap()
    ssin2T = nc.dram_tensor("ssin2T", [64, S], BF, kind="ExternalInput").ap()
    maskD = nc.dram_tensor("maskD", [128, 256], BF, kind="ExternalInput").ap()
    pmat = nc.dram_tensor("pmat", [128, 128], BF, kind="ExternalInput").ap()
    outT = nc.dram_tensor("outT", [128, NQC * 8 * QC], BF, kind="ExternalOutput").ap()

    with tile.TileContext(nc) as tc:
        build_tile_program(tc, hsT, wqkT, wvT, woT, cos2T, ssin2T, maskD, pmat, outT)
    nc.compile()
    return nc


def build_tile_program(tc, hsT, wqkT, wvT, woT, cos2T, ssin2T, maskD, pmat, outT):
    nc = tc.nc
    with ExitStack() as ctx:
        const = ctx.enter_context(tc.tile_pool(name="const", bufs=1))
        persist = ctx.enter_context(tc.tile_pool(name="persist", bufs=1))
        work = ctx.enter_context(tc.tile_pool(name="work", bufs=5))
        posbp = ctx.enter_context(tc.tile_pool(name="posbp", bufs=6))
        expp = ctx.enter_context(tc.tile_pool(name="expp", bufs=8))
        small = ctx.enter_context(tc.tile_pool(name="small", bufs=4))
        ps_main = ctx.enter_context(tc.tile_pool(name="ps_main", bufs=2, space="PSUM"))
        ps_sc = ctx.enter_context(tc.tile_pool(name="ps_sc", bufs=2, space="PSUM"))
        ps_po = ctx.enter_context(tc.tile_pool(name="ps_po", bufs=2, space="PSUM"))

        # ---- SBUF destination tiles ----
        wqk_sb = const.tile([128, KHID, 512], BF, name="wqk_sb")
        hs_sb = const.tile([128, NQC, KHID, QC], BF, name="hs_sb")
        cos_sb = const.tile([128, S], BF, name="cos_sb")
        ssin_sb = const.tile([128, S], BF, name="ssin_sb")
        tri_sb = const.tile([128, 2, 128], BF, name="tri_sb")
        pmat_sb = const.tile([128, 128], BF, name="pmat_sb")
        wv_sb = const.tile([128, KHID, 256], BF, name="wv_sb")
        wo_sb = const.tile([128, 2, HID], BF, name="wo_sb")

        # ---- input DMAs: kk-granular, spread over four engine queues, in
        # priority order (wqk + hs chunk0 first so the first matmul can start
        # ~3us in; later chunks stream behind the compute). ----
        def hs_piece(t, k2):  # kk pair [2k2, 2k2+2) of chunk t
            return (
                hs_sb[:, t, 2 * k2:2 * k2 + 2, :],
                hsT[:, t * KHID * QC + 2 * k2 * QC: t * KHID * QC + (2 * k2 + 2) * QC]
                .rearrange("p (k s) -> p k s", k=2),
            )

        def wqk_piece(k2):
            return (
                wqk_sb[:, 2 * k2:2 * k2 + 2, :],
                wqkT[:, 2 * k2 * 512:(2 * k2 + 2) * 512].rearrange("p (k m) -> p k m", k=2),
            )

        # warm the ACT table (exp set loads ~2.7us) while DMAs stream
        warm = const.tile([1, 16], FP, name="warm")
        nc.scalar.activation(warm[:], warm[:], EXP, scale=0.0)

        def hs_half(t, h):  # kk half [4h, 4h+4) of chunk t
            return (
                hs_sb[:, t, 4 * h:4 * h + 4, :],
                hsT[:, t * KHID * QC + 4 * h * QC: t * KHID * QC + (4 * h + 4) * QC]
                .rearrange("p (k s) -> p k s", k=4),
            )

        def wqk_one(k):
            return (wqk_sb[:, k, :], wqkT[:, k * 512:(k + 1) * 512])

        def hs_one(t, k):
            return (hs_sb[:, t, k, :],
                    hsT[:, t * KHID * QC + k * QC: t * KHID * QC + (k + 1) * QC])

        qS, qG, qC = nc.sync, nc.gpsimd, nc.scalar
        plan = [
            (qS, wqk_one(0)), (qG, hs_one(0, 0)),
            (qS, wqk_one(1)), (qG, hs_one(0, 1)),
            (qC, hs_piece(0, 1)),
            (qS, wqk_piece(1)), (qG, hs_piece(0, 2)), (qC, hs_piece(0, 3)),
            (qS, wqk_piece(2)), (qG, hs_piece(1, 0)), (qC, (pmat_sb[:], pmat[:])),
            (qS, wqk_piece(3)), (qG, hs_piece(1, 1)),
            (qC, (cos_sb[0:64, :], cos2T[:])),
            (qC, (cos_sb[64:128, :], cos2T[:])),
            (qS, hs_piece(1, 2)), (qG, hs_piece(1, 3)),
            (qC, (ssin_sb[0:64, :], ssin2T[:])),
            (qC, (ssin_sb[64:128, :], ssin2T[:])),
            (qS, (tri_sb[:], maskD.rearrange("p (r c) -> p r c", r=2))),
            (qG, (wv_sb[:, 0:4, :], wvT[:, 0:1024].rearrange("p (k m) -> p k m", k=4))),
            (qS, (wv_sb[:, 4:8, :], wvT[:, 1024:2048].rearrange("p (k m) -> p k m", k=4))),
            (qG, hs_half(2, 0)), (qS, hs_half(2, 1)),
            (qC, (wo_sb[:], woT.rearrange("p (k m) -> p k m", k=2))),
            (qG, hs_half(3, 0)), (qS, hs_half(3, 1)),
        ]
        for eng, (dst, src) in plan:
            eng.dma_start(dst, src)

        # HAM warm-up: ~24 back-to-back matmuls over the first-landed pieces.
        # Re-warming to 2.4 GHz needs a sustained burst of PE activity; the
        # DMA-bound start is otherwise too choppy to trigger it, leaving the
        # PE at 1.2 GHz well into the attention phase. Result is discarded.
        wps = ps_main.tile([128, QC], FP, name="warm_ps", tag="ps")
        for _ in range(24):
            nc.tensor.matmul(wps[:], wqk_sb[:, 0, 0:128], hs_sb[:, 0, 0, :],
                             start=True, stop=True)

        qkT = persist.tile([128, 4, S], BF, name="qkT")
        v_sb = persist.tile([128, NKT, 4 * 65], BF, name="v_sb2")
        att_sb = persist.tile([128, 2, S], BF, name="att_sb2")
        nc.vector.memset(
            v_sb.rearrange("p t (h c) -> p t h c", c=65)[:, :, :, 64:65], 1.0
        )

        # ---- work units (each emits a bounded slug of PE work + its
        # elementwise tail on DVE/pool, or ACT in the DMA-bound pre-phase) ----

        def proj_a(rb, t):
            # first half of the qk projection for (rb, t): 4 contraction steps
            ps = ps_main.tile([128, QC], FP, name="ps_qk", tag="ps")
            for kk in range(4):
                nc.tensor.matmul(
                    ps[:], wqk_sb[:, kk, rb * 128:(rb + 1) * 128],
                    hs_sb[:, t, kk, :], start=(kk == 0), stop=False,
                )
            return ps

        def proj_b(rb, t, ps, act=False):
            # second half + RoPE
            csl = slice(t * QC, (t + 1) * QC)
            for kk in range(4, KHID):
                nc.tensor.matmul(
                    ps[:], wqk_sb[:, kk, rb * 128:(rb + 1) * 128],
                    hs_sb[:, t, kk, :], start=False, stop=(kk == KHID - 1),
                )
            x = work.tile([128, QC], BF, name="x_rope", tag="xrope")
            if act:
                nc.scalar.copy(x[:], ps[:])
            else:
                nc.vector.tensor_copy(x[:], ps[:])
            xs_ps = ps_main.tile([128, QC], FP, name="xs_ps", tag="ps")
            nc.tensor.matmul(xs_ps[:], pmat_sb[:], x[:], start=True, stop=True)
            t1 = work.tile([128, QC], BF, name="t1_rope", tag="t1rope")
            t2 = work.tile([128, QC], BF, name="t2_rope", tag="t2rope")
            nc.vector.tensor_mul(t1[:], x[:], cos_sb[:, csl])
            nc.vector.tensor_mul(t2[:], xs_ps[:], ssin_sb[:, csl])
            nc.vector.tensor_add(qkT[:, rb, csl], t1[:], t2[:])

        def v_unit(tt, act=False):
            psv = ps_main.tile([128, 256], FP, name="ps_v", tag="ps")
            for kk in range(KHID):
                nc.tensor.matmul(
                    psv[:],
                    hs_sb[:, tt // 4, kk, (tt % 4) * 128:(tt % 4 + 1) * 128],
                    wv_sb[:, kk, :], start=(kk == 0), stop=(kk == KHID - 1),
                )
            dst = v_sb[:, tt, :].rearrange("p (h c) -> p h c", c=65)[:, :, 0:64]
            src = psv[:].rearrange("p (h c) -> p h c", c=64)
            if act:
                nc.scalar.copy(dst, src)
            else:
                nc.vector.tensor_copy(dst, src)

        out_q = [nc.sync, nc.gpsimd]

        def oproj_ot(qi, half, oi, ow, act=False):
            qsl = slice(qi * QC, (qi + 1) * QC)
            ot = half * 4 + oi
            pw = ps_main.tile([128, QC], FP, name="pw", tag="ps")
            for p in range(2):
                nc.tensor.matmul(
                    pw[:], wo_sb[:, p, ot * 128:(ot + 1) * 128],
                    att_sb[:, p, qsl], start=(p == 0), stop=(p == 1),
                )
            if act or oi % 2 == 1:
                nc.scalar.copy(ow[:, oi, :], pw[:])
            else:
                nc.vector.tensor_copy(ow[:, oi, :], pw[:])
            if oi % 2 == 1:
                off = (qi * 2 + half) * 4 * QC + (oi - 1) * QC
                eng = out_q[(qi * 2 + half + oi) % 2]
                eng.dma_start(
                    outT[:, off:off + 2 * QC].rearrange("p (o s) -> p o s", o=2),
                    ow[:, oi - 1:oi + 1, :],
                )

        # filler machinery: (name, cost_ns, closure). Emission order == PE FIFO
        # order, so any attention matmul must be emitted after every unit it
        # reads from; require() flushes the deque up to a named unit. pump()
        # spends an accumulated credit so filler emission is paced smoothly
        # (bursts in the PE FIFO starve ACT, the attention rate-setter).
        fillers = deque()
        emitted = set()
        credit = [0]

        def pump(budget):
            while budget > 0 and fillers:
                name, cost, fn = fillers.popleft()
                fn()
                emitted.add(name)
                budget -= cost

        def require(*names):
            while any(n not in emitted for n in names) and fillers:
                name, cost, fn = fillers.popleft()
                fn()
                emitted.add(name)

        def push_proj(rb, t):
            st = {}
            fillers.append((f"pa{rb}_{t}", 900, lambda: st.__setitem__("ps", proj_a(rb, t))))
            fillers.append((f"p{rb}_{t}", 1100, lambda: proj_b(rb, t, st["ps"])))

        def push_v(t):
            for tt in range(4 * t, 4 * t + 4):
                fillers.append((f"v{tt}", 1000, lambda tt=tt: v_unit(tt)))

        def push_oproj(qi, act_all=False):
            st = {}
            for half in range(2):
                def mk(half):
                    def alloc():
                        st[half] = work.tile([128, 4, QC], BF, name="ow", tag="ow")
                    return alloc
                alloc = mk(half)
                for oi in range(4):
                    def u(half=half, oi=oi, alloc=alloc):
                        if oi == 0:
                            alloc()
                        oproj_ot(qi, half, oi, st[half], act=act_all)
                    fillers.append((f"o{qi}_{half}_{oi}", 650, u))

        # ---- attention ----

        def attention_unit(pair, qi):
            """Emits scores^T -> exp -> PV for heads (2*pair, 2*pair+1) of query
            chunk qi, with filler units threaded between QK(ki) and PV(ki-1) so
            the PE keeps working while ACT runs the exp. Returns (po0, po1)."""
            qsl = slice(qi * QC, (qi + 1) * QC)
            nki = 4 * qi + 4
            po0 = ps_po.tile([65, QC], FP, name="po0", tag="po")
            po1 = ps_po.tile([65, QC], FP, name="po1", tag="po")
            pend = None  # (ki, e, lo) awaiting PV

            def emit_pv(ki, e, lo):
                h0 = 2 * pair
                h1 = 2 * pair + 1
                nc.tensor.matmul(
                    po0[:, lo:QC], v_sb[:, ki, h0 * 65:(h0 + 1) * 65], e[:, 0, lo:QC],
                    start=(ki == 0), stop=(ki == nki - 1),
                )
                nc.tensor.matmul(
                    po1[:, lo:QC], v_sb[:, ki, h1 * 65:(h1 + 1) * 65], e[:, 1, lo:QC],
                    start=(ki == 0), stop=(ki == nki - 1),
                )

            require(f"p{pair}_{qi}", f"p{2 + pair}_0")
            for ki in range(nki):
                ksl = slice(ki * KT, (ki + 1) * KT)
                j = ki - 4 * qi
                lo = 0 if j < 0 else 128 * j  # first live q column in this tile
                qslh = slice(qi * QC + lo, (qi + 1) * QC)
                if ki % 4 == 0:
                    require(f"p{2 + pair}_{ki // 4}")
                psc = ps_sc.tile([128, 2, QC], FP, name="psc", tag="sc")
                nc.tensor.matmul(
                    psc[:, 0, lo:QC], qkT[0:64, 2 + pair, ksl], qkT[0:64, pair, qslh],
                    start=True, stop=True,
                )
                nc.tensor.matmul(
                    psc[:, 1, lo:QC], qkT[64:128, 2 + pair, ksl], qkT[64:128, pair, qslh],
                    start=True, stop=True,
                )
                e = expp.tile([128, 2, QC], BF, name="e", tag="exp")
                if lo == 0:
                    # contiguous flat view: one run on ACT instead of two rows
                    nc.scalar.activation(
                        e.rearrange("p a b -> p (a b)")[:, :],
                        psc.rearrange("p a b -> p (a b)")[:, :], EXP, scale=0.125,
                    )
                else:
                    nc.scalar.activation(
                        e[:, :, lo:QC], psc[:, :, lo:QC], EXP, scale=0.125
                    )
                if j >= 0:
                    nc.vector.tensor_mul(
                        e[:, :, lo:lo + 128], e[:, :, lo:lo + 128], tri_sb[:]
                    )
                if pend is not None:
                    pump(400 if j < 1 else 150)
                    require(f"v{pend[0]}")
                    emit_pv(*pend)
                pend = (ki, e, lo)
            pump(150)
            require(f"v{pend[0]}")
            emit_pv(*pend)
            # free the po banks promptly: numerators+denominator to SBUF via
            # ACT, which is idle exactly at these unit boundaries (no exps);
            # the normalization then runs off the PSUM-release path
            po_sb0 = posbp.tile([65, QC], FP, name="po_sb0", tag="posb")
            po_sb1 = posbp.tile([65, QC], FP, name="po_sb1", tag="posb")
            nc.scalar.copy(po_sb0[:], po0[:])
            nc.scalar.copy(po_sb1[:], po1[:])
            return po_sb0, po_sb1

        def division(pair, qi, po_sb0, po_sb1):
            """att = po / l from the SBUF copies, off the PSUM-release path."""
            qsl = slice(qi * QC, (qi + 1) * QC)
            for sub, posb in enumerate([po_sb0, po_sb1]):
                l_sb = small.tile([1, QC], FP, name="l_sb", tag="lsb", bufs=4)
                nc.vector.tensor_copy(l_sb[:], posb[64:65, :])
                rl = small.tile([1, QC], FP, name="rl", tag="rl", bufs=4)
                nc.vector.reciprocal_approx_fast(out=rl[:], in_=l_sb[:])
                rb_ = small.tile([64, QC], FP, name="rb_", tag="rbb", bufs=4)
                nc.gpsimd.partition_broadcast(rb_[:], rl[:])
                nc.vector.tensor_mul(
                    att_sb[sub * 64:(sub + 1) * 64, pair, qsl], posb[0:64, :], rb_[:]
                )

        # ---- emission schedule ----
        # pre-phase (DMA-bound): just the two units the first attention QKs
        # need, with PSUM->SBUF copies on the otherwise-idle ACT engine;
        # everything else becomes paced filler, flushed on demand by the
        # fine-grained require() calls inside attention_unit.
        for rb, t in [(2, 0), (0, 1)]:
            ps = proj_a(rb, t)
            proj_b(rb, t, ps, act=True)
            emitted.add(f"p{rb}_{t}")
        for tt in range(4):
            v_unit(tt, act=True)
            emitted.add(f"v{tt}")

        push_proj(2, 1)
        for tt in range(4, 8):
            fillers.append((f"v{tt}", 1000, lambda tt=tt: v_unit(tt)))
        push_proj(3, 0); push_proj(1, 1); push_proj(3, 1)        # att(1, p1)
        push_proj(0, 0); push_proj(1, 0)                         # att(0)
        push_proj(0, 2); push_proj(2, 2); push_v(2)              # att(2, p0)
        push_proj(1, 2); push_proj(3, 2)                         # att(2, p1)
        push_proj(0, 3); push_proj(2, 3); push_v(3)              # att(3, p0)
        push_proj(1, 3); push_proj(3, 3)                         # att(3, p1)

        order = [1, 2, 3, 0]
        for qi in order:
            for pair in range(2):
                pos = attention_unit(pair, qi)
                division(pair, qi, *pos)
            if qi != order[-1]:
                push_oproj(qi)

        # drain leftovers, then the final o_proj chunk with copies alternating
        # between the now-idle ACT and DVE, per-ot DMAs over three queues
        require(*[n for n, _, _ in list(fillers)])
        ql = order[-1]
        tail_q = [nc.sync, nc.gpsimd, nc.scalar]
        for half in range(2):
            ow = work.tile([128, 4, QC], BF, name="ow", tag="ow")
            for oi in range(4):
                qsl = slice(ql * QC, (ql + 1) * QC)
                ot = half * 4 + oi
                pw = ps_main.tile([128, QC], FP, name="pw", tag="ps")
                for p in range(2):
                    nc.tensor.matmul(
                        pw[:], wo_sb[:, p, ot * 128:(ot + 1) * 128],
                        att_sb[:, p, qsl], start=(p == 0), stop=(p == 1),
                    )
                if oi % 2 == 0:
                    nc.scalar.copy(ow[:, oi, :], pw[:])
                else:
                    nc.vector.tensor_copy(ow[:, oi, :], pw[:])
                off = (ql * 2 + half) * 4 * QC + oi * QC
                tail_q[(half * 4 + oi) % 3].dma_start(
                    outT[:, off:off + QC], ow[:, oi, :],
                )


# ---------- host-side shard preparation ----------

def make_core_inputs(hidden_states, cos, sin, w_qkv, w_o):
    """Returns list of 8 in_maps (numpy, bf16 where needed)."""
    bf = ml_dtypes.bfloat16
    hs = np.asarray(hidden_states, np.float32)
    cos = np.asarray(cos, np.float32)
    sin = np.asarray(sin, np.float32)
    w_qkv = np.asarray(w_qkv, np.float32)
    w_o = np.asarray(w_o, np.float32)

    cosT = cos.T
    sinT = sin.T
    cos2T = np.ascontiguousarray(cosT).astype(bf)
    ssin2T = np.ascontiguousarray(sinT).astype(bf)
    # signed rotate-half permutation: out[m] = sign(m) * x[partner(m)]
    # lhsT layout: pmat[k, m] = sign(m) at k = partner(m)
    pmat = np.zeros((128, 128), np.float32)
    for m in range(128):
        d = m % 64
        base = m - d
        if d < 32:
            pmat[base + d + 32, m] = -1.0
        else:
            pmat[base + d - 32, m] = 1.0
    pmat = pmat.astype(bf)

    kp = np.arange(128)[:, None]
    cc = np.arange(128)[None, :]
    tri = (kp <= cc).astype(bf)
    maskD = np.concatenate([tri, tri], axis=1)

    def swz(a):
        # [K*128, M] -> [128, K*M] partition-major (matches SBUF tiles)
        k = a.shape[0] // 128
        return np.ascontiguousarray(
            a.reshape(k, 128, a.shape[1]).transpose(1, 0, 2).reshape(128, -1))

    in_maps = []
    for c in range(8):
        b, g = divmod(c, 4)
        heads = range(4 * g, 4 * g + 4)
        hsT = np.ascontiguousarray(hs[b].T).astype(bf)  # [HID, S]
        # chunk-major swizzle: [128, t, kk, s']
        hs2 = np.ascontiguousarray(
            hsT.reshape(8, 128, 4, 512).transpose(1, 2, 0, 3).reshape(128, -1))
        wq = np.concatenate([w_qkv[h * 64:(h + 1) * 64] for h in heads], 0)
        wk = np.concatenate([w_qkv[HID + h * 64:HID + (h + 1) * 64] for h in heads], 0)
        wv = np.concatenate([w_qkv[2 * HID + h * 64:2 * HID + (h + 1) * 64] for h in heads], 0)
        wqkT = swz(np.ascontiguousarray(np.concatenate([wq, wk], 0).T).astype(bf))
        wvT = swz(np.ascontiguousarray(wv.T).astype(bf))
        woT = swz(np.ascontiguousarray(
            np.concatenate([w_o[:, h * 64:(h + 1) * 64] for h in heads], 1).T
        ).astype(bf))
        in_maps.append({
            "hsT": hs2, "wqkT": wqkT, "wvT": wvT, "woT": woT,
            "cos2T": cos2T, "ssin2T": ssin2T, "maskD": maskD, "pmat": pmat,
        })
    return in_maps


def unswizzle_out(o2):
    # [128, qi*half*oi*512] -> outT [1024, 2048]
    a = o2.reshape(128, NQC, 2, 4, QC)
    return np.ascontiguousarray(
        a.transpose(2, 3, 0, 1, 4).reshape(HID, S))


def unshard(outTs):
    out = np.zeros((B, S, HID), np.float32)
    for c, o2 in enumerate(outTs):
        out[c // 4] += unswizzle_out(o2).T.astype(np.float32)
    return out


# ---------- standalone kernel entry ----------

from concourse.bass_utils import run_bass_kernel_spmd

_CACHED_NC = None


def get_program():
    global _CACHED_NC
    if _CACHED_NC is None:
        _CACHED_NC = build_program()
    return _CACHED_NC


def run(inputs, trace=False):
    nc = get_program()
    in_maps = make_core_inputs(**inputs)
    res = run_bass_kernel_spmd(nc, in_maps, core_ids=list(range(8)), trace=trace)
    out = np.zeros((B, S, HID), np.float32)
    for c, r in enumerate(res.results):
        out[c // 4] += unswizzle_out(r["outT"]).T.astype(np.float32)
    return out, res


def kernel(**inputs):
    out, _ = run(inputs, trace=False)
    return out


# revision 34
# speedup vs baseline: 1.2044x; 1.0013x over previous
"""Sharded causal-attention kernel for 8 trn2 NeuronCores.

DP over batch (2) x TP over head groups (4 heads/core). Each core: qkv projection
(its heads) + RoPE + causal SDPA (scores kept transposed; softmax denominator via a
ones-column in the PV matmul) + its 256-row slice of the o_proj contraction, returning
a transposed partial [HID, S]; the host sums 4 partials per batch. bf16 matmuls,
fp32 PSUM accumulation.

Schedule: the attention inner loop is exp(ACT)-gated, so projection / v / o_proj
work is emitted as small "filler" units threaded between the QK and PV matmuls of
each key tile, keeping the PE FIFO saturated (and its clock un-throttled) while
ACT grinds exps. Input DMAs are kk-granular across the three DMA-capable engine
queues (sync/gpsimd/scalar) in priority order; a 24-matmul warm-up burst over the
first-landed piece flips the PE clock gate to full rate during the DMA-bound
start. PV numerators+denominator are copied out of PSUM on ACT (idle at unit
boundaries) so the banks recycle without waiting on the reciprocal/broadcast
chain; output DMAs are issued per-pair-of-columns-tiles and rotate across queues
so the store drains while compute proceeds.
"""

import sys

sys.path.insert(0, "/opt/trn_rl_repo")

from collections import deque
from contextlib import ExitStack

import numpy as np
import ml_dtypes

import concourse.bass as bass
import concourse.mybir as mybir
import concourse.tile as tile
from concourse import bacc

FP = mybir.dt.float32
BF = mybir.dt.bfloat16
EXP = mybir.ActivationFunctionType.Exp

B, S, HID = 2, 2048, 1024
H, D = 16, 64
QC = 512
KT = 128
NQC = S // QC
NKT = S // KT
KHID = HID // 128


def build_program(debug_outputs=False):
    nc = bacc.Bacc("TRN2", target_bir_lowering=False, debug=False, num_devices=8, num_swdge_queues=2)

    hsT = nc.dram_tensor("hsT", [128, NQC * KHID * QC], BF, kind="ExternalInput").ap()
    wqkT = nc.dram_tensor("wqkT", [128, KHID * 512], BF, kind="ExternalInput").ap()
    wvT = nc.dram_tensor("wvT", [128, KHID * 256], BF, kind="ExternalInput").ap()
    woT = nc.dram_tensor("woT", [128, 2 * HID], BF, kind="ExternalInput").ap()
    cos2T = nc.dram_tensor("cos2T", [64, S], BF, kind="ExternalInput").ap()
    ssin2T = nc.dram_tensor("ssin2T", [64, S], BF, kind="ExternalInput").ap()
    maskD = nc.dram_tensor("maskD", [128, 256], BF, kind="ExternalInput").ap()
    pmat = nc.dram_tensor("pmat", [128, 128], BF, kind="ExternalInput").ap()
    outT = nc.dram_tensor("outT", [128, NQC * 8 * QC], BF, kind="ExternalOutput").ap()

    with tile.TileContext(nc) as tc:
        build_tile_program(tc, hsT, wqkT, wvT, woT, cos2T, ssin2T, maskD, pmat, outT)
    nc.compile()
    return nc


def build_tile_program(tc, hsT, wqkT, wvT, woT, cos2T, ssin2T, maskD, pmat, outT):
    nc = tc.nc
    with ExitStack() as ctx:
        const = ctx.enter_context(tc.tile_pool(name="const", bufs=1))
        persist = ctx.enter_context(tc.tile_pool(name="persist", bufs=1))
        work = ctx.enter_context(tc.tile_pool(name="work", bufs=5))
        posbp = ctx.enter_context(tc.tile_pool(name="posbp", bufs=6))
        expp = ctx.enter_context(tc.tile_pool(name="expp", bufs=8))
        small = ctx.enter_context(tc.tile_pool(name="small", bufs=4))
        ps_main = ctx.enter_context(tc.tile_pool(name="ps_main", bufs=2, space="PSUM"))
        ps_sc = ctx.enter_context(tc.tile_pool(name="ps_sc", bufs=2, space="PSUM"))
        ps_po = ctx.enter_context(tc.tile_pool(name="ps_po", bufs=2, space="PSUM"))

        # ---- SBUF destination tiles ----
        wqk_sb = const.tile([128, KHID, 512], BF, name="wqk_sb")
        hs_sb = const.tile([128, NQC, KHID, QC], BF, name="hs_sb")
        cos_sb = const.tile([128, S], BF, name="cos_sb")
        ssin_sb = const.tile([128, S], BF, name="ssin_sb")
        tri_sb = const.tile([128, 2, 128], BF, name="tri_sb")
        pmat_sb = const.tile([128, 128], BF, name="pmat_sb")
        wv_sb = const.tile([128, KHID, 256], BF, name="wv_sb")
        wo_sb = const.tile([128, 2, HID], BF, name="wo_sb")

        # ---- input DMAs: kk-granular, spread over four engine queues, in
        # priority order (wqk + hs chunk0 first so the first matmul can start
        # ~3us in; later chunks stream behind the compute). ----
        def hs_piece(t, k2):  # kk pair [2k2, 2k2+2) of chunk t
            return (
                hs_sb[:, t, 2 * k2:2 * k2 + 2, :],
                hsT[:, t * KHID * QC + 2 * k2 * QC: t * KHID * QC + (2 * k2 + 2) * QC]
                .rearrange("p (k s) -> p k s", k=2),
            )

        def wqk_piece(k2):
            return (
                wqk_sb[:, 2 * k2:2 * k2 + 2, :],
                wqkT[:, 2 * k2 * 512:(2 * k2 + 2) * 512].rearrange("p (k m) -> p k m", k=2),
            )

        # warm the ACT table (exp set loads ~2.7us) while DMAs stream
        warm = const.tile([1, 16], FP, name="warm")
        nc.scalar.activation(warm[:], warm[:], EXP, scale=0.0)

        def hs_half(t, h):  # kk half [4h, 4h+4) of chunk t
            return (
                hs_sb[:, t, 4 * h:4 * h + 4, :],
                hsT[:, t * KHID * QC + 4 * h * QC: t * KHID * QC + (4 * h + 4) * QC]
                .rearrange("p (k s) -> p k s", k=4),
            )

        def wqk_one(k):
            return (wqk_sb[:, k, :], wqkT[:, k * 512:(k + 1) * 512])

        def hs_one(t, k):
            return (hs_sb[:, t, k, :],
                    hsT[:, t * KHID * QC + k * QC: t * KHID * QC + (k + 1) * QC])

        qS, qG, qC = nc.sync, nc.gpsimd, nc.scalar
        plan = [
            (qS, wqk_one(0)), (qG, hs_one(0, 0)),
            (qS, wqk_one(1)), (qG, hs_one(0, 1)),
            (qC, hs_piece(0, 1)),
            (qS, wqk_piece(1)), (qG, hs_piece(0, 2)), (qC, hs_piece(0, 3)),
            (qS, wqk_piece(2)), (qG, hs_piece(1, 0)), (qC, (pmat_sb[:], pmat[:])),
            (qS, wqk_piece(3)), (qG, hs_piece(1, 1)),
            (qC, (cos_sb[0:64, :], cos2T[:])),
            (qC, (cos_sb[64:128, :], cos2T[:])),
            (qS, hs_piece(1, 2)), (qG, hs_piece(1, 3)),
            (qC, (ssin_sb[0:64, :], ssin2T[:])),
            (qC, (ssin_sb[64:128, :], ssin2T[:])),
            (qS, (tri_sb[:], maskD.rearrange("p (r c) -> p r c", r=2))),
            (qG, (wv_sb[:, 0:4, :], wvT[:, 0:1024].rearrange("p (k m) -> p k m", k=4))),
            (qS, (wv_sb[:, 4:8, :], wvT[:, 1024:2048].rearrange("p (k m) -> p k m", k=4))),
            (qG, hs_half(2, 0)), (qS, hs_half(2, 1)),
            (qC, (wo_sb[:], woT.rearrange("p (k m) -> p k m", k=2))),
            (qG, hs_half(3, 0)), (qS, hs_half(3, 1)),
        ]
        for eng, (dst, src) in plan:
            eng.dma_start(dst, src)

        # HAM warm-up: ~24 back-to-back matmuls over the first-landed pieces.
        # Re-warming to 2.4 GHz needs a sustained burst of PE activity; the
        # DMA-bound start is otherwise too choppy to trigger it, leaving the
        # PE at 1.2 GHz well into the attention phase. Result is discarded.
        wps = ps_main.tile([128, QC], FP, name="warm_ps", tag="ps")
        for _ in range(24):
            nc.tensor.matmul(wps[:], wqk_sb[:, 0, 0:128], hs_sb[:, 0, 0, :],
                             start=True, stop=True)

        qkT = persist.tile([128, 4, S], BF, name="qkT")
        v_sb = persist.tile([128, NKT, 4 * 65], BF, name="v_sb2")
        att_sb = persist.tile([128, 2, S], BF, name="att_sb2")
        nc.vector.memset(
            v_sb.rearrange("p t (h c) -> p t h c", c=65)[:, :, :, 64:65], 1.0
        )

        # ---- work units (each emits a bounded slug of PE work + its
        # elementwise tail on DVE/pool, or ACT in the DMA-bound pre-phase) ----

        def proj_a(rb, t):
            # first half of the qk projection for (rb, t): 4 contraction steps
            ps = ps_main.tile([128, QC], FP, name="ps_qk", tag="ps")
            for kk in range(4):
                nc.tensor.matmul(
                    ps[:], wqk_sb[:, kk, rb * 128:(rb + 1) * 128],
                    hs_sb[:, t, kk, :], start=(kk == 0), stop=False,
                )
            return ps

        def proj_b(rb, t, ps, act=False):
            # second half + RoPE
            csl = slice(t * QC, (t + 1) * QC)
            for kk in range(4, KHID):
                nc.tensor.matmul(
                    ps[:], wqk_sb[:, kk, rb * 128:(rb + 1) * 128],
                    hs_sb[:, t, kk, :], start=False, stop=(kk == KHID - 1),
                )
            x = work.tile([128, QC], BF, name="x_rope", tag="xrope")
            if act:
                nc.scalar.copy(x[:], ps[:])
            else:
                nc.vector.tensor_copy(x[:], ps[:])
            xs_ps = ps_main.tile([128, QC], FP, name="xs_ps", tag="ps")
            nc.tensor.matmul(xs_ps[:], pmat_sb[:], x[:], start=True, stop=True)
            t1 = work.tile([128, QC], BF, name="t1_rope", tag="t1rope")
            t2 = work.tile([128, QC], BF, name="t2_rope", tag="t2rope")
            nc.vector.tensor_mul(t1[:], x[:], cos_sb[:, csl])
            nc.vector.tensor_mul(t2[:], xs_ps[:], ssin_sb[:, csl])
            nc.vector.tensor_add(qkT[:, rb, csl], t1[:], t2[:])

        def v_unit(tt, act=False):
            psv = ps_main.tile([128, 256], FP, name="ps_v", tag="ps")
            for kk in range(KHID):
                nc.tensor.matmul(
                    psv[:],
                    hs_sb[:, tt // 4, kk, (tt % 4) * 128:(tt % 4 + 1) * 128],
                    wv_sb[:, kk, :], start=(kk == 0), stop=(kk == KHID - 1),
                )
            dst = v_sb[:, tt, :].rearrange("p (h c) -> p h c", c=65)[:, :, 0:64]
            src = psv[:].rearrange("p (h c) -> p h c", c=64)
            if act:
                nc.scalar.copy(dst, src)
            else:
                nc.vector.tensor_copy(dst, src)

        out_q = [nc.sync, nc.gpsimd]

        def oproj_ot(qi, half, oi, ow, act=False):
            qsl = slice(qi * QC, (qi + 1) * QC)
            ot = half * 4 + oi
            pw = ps_main.tile([128, QC], FP, name="pw", tag="ps")
            for p in range(2):
                nc.tensor.matmul(
                    pw[:], wo_sb[:, p, ot * 128:(ot + 1) * 128],
                    att_sb[:, p, qsl], start=(p == 0), stop=(p == 1),
                )
            if act or oi % 2 == 1:
                nc.scalar.copy(ow[:, oi, :], pw[:])
            else:
                nc.vector.tensor_copy(ow[:, oi, :], pw[:])
            if oi % 2 == 1:
                off = (qi * 2 + half) * 4 * QC + (oi - 1) * QC
                eng = out_q[(qi * 2 + half + oi) % 2]
                eng.dma_start(
                    outT[:, off:off + 2 * QC].rearrange("p (o s) -> p o s", o=2),
                    ow[:, oi - 1:oi + 1, :],
                )

        # filler machinery: (name, cost_ns, closure). Emission order == PE FIFO
        # order, so any attention matmul must be emitted after every unit it
        # reads from; require() flushes the deque up to a named unit. pump()
        # spends an accumulated credit so filler emission is paced smoothly
        # (bursts in the PE FIFO starve ACT, the attention rate-setter).
        fillers = deque()
        emitted = set()
        credit = [0]

        def pump(budget):
            while budget > 0 and fillers:
                name, cost, fn = fillers.popleft()
                fn()
                emitted.add(name)
                budget -= cost

        def require(*names):
            while any(n not in emitted for n in names) and fillers:
                name, cost, fn = fillers.popleft()
                fn()
                emitted.add(name)

        def push_proj(rb, t):
            st = {}
            fillers.append((f"pa{rb}_{t}", 900, lambda: st.__setitem__("ps", proj_a(rb, t))))
            fillers.append((f"p{rb}_{t}", 1100, lambda: proj_b(rb, t, st["ps"])))

        def push_v(t):
            for tt in range(4 * t, 4 * t + 4):
                fillers.append((f"v{tt}", 1000, lambda tt=tt: v_unit(tt)))

        def push_oproj(qi, act_all=False):
            st = {}
            for half in range(2):
                def mk(half):
                    def alloc():
                        st[half] = work.tile([128, 4, QC], BF, name="ow", tag="ow")
                    return alloc
                alloc = mk(half)
                for oi in range(4):
                    def u(half=half, oi=oi, alloc=alloc):
                        if oi == 0:
                            alloc()
                        oproj_ot(qi, half, oi, st[half], act=act_all)
                    fillers.append((f"o{qi}_{half}_{oi}", 650, u))

        # ---- attention ----

        def attention_unit(pair, qi):
            """Emits scores^T -> exp -> PV for heads (2*pair, 2*pair+1) of query
            chunk qi, with filler units threaded between QK(ki) and PV(ki-1) so
            the PE keeps working while ACT runs the exp. Returns (po0, po1)."""
            qsl = slice(qi * QC, (qi + 1) * QC)
            nki = 4 * qi + 4
            po0 = ps_po.tile([65, QC], FP, name="po0", tag="po")
            po1 = ps_po.tile([65, QC], FP, name="po1", tag="po")
            pend = None  # (ki, e, lo) awaiting PV

            def emit_pv(ki, e, lo):
                h0 = 2 * pair
                h1 = 2 * pair + 1
                nc.tensor.matmul(
                    po0[:, lo:QC], v_sb[:, ki, h0 * 65:(h0 + 1) * 65], e[:, 0, lo:QC],
                    start=(ki == 0), stop=(ki == nki - 1),
                )
                nc.tensor.matmul(
                    po1[:, lo:QC], v_sb[:, ki, h1 * 65:(h1 + 1) * 65], e[:, 1, lo:QC],
                    start=(ki == 0), stop=(ki == nki - 1),
                )

            require(f"p{pair}_{qi}", f"p{2 + pair}_0")
            for ki in range(nki):
                ksl = slice(ki * KT, (ki + 1) * KT)
                j = ki - 4 * qi
                lo = 0 if j < 0 else 128 * j  # first live q column in this tile
                qslh = slice(qi * QC + lo, (qi + 1) * QC)
                if ki % 4 == 0:
                    require(f"p{2 + pair}_{ki // 4}")
                psc = ps_sc.tile([128, 2, QC], FP, name="psc", tag="sc")
                nc.tensor.matmul(
                    psc[:, 0, lo:QC], qkT[0:64, 2 + pair, ksl], qkT[0:64, pair, qslh],
                    start=True, stop=True,
                )
                nc.tensor.matmul(
                    psc[:, 1, lo:QC], qkT[64:128, 2 + pair, ksl], qkT[64:128, pair, qslh],
                    start=True, stop=True,
                )
                e = expp.tile([128, 2, QC], BF, name="e", tag="exp")
                if lo == 0:
                    # contiguous flat view: one run on ACT instead of two rows
                    nc.scalar.activation(
                        e.rearrange("p a b -> p (a b)")[:, :],
                        psc.rearrange("p a b -> p (a b)")[:, :], EXP, scale=0.125,
                    )
                else:
                    nc.scalar.activation(
                        e[:, :, lo:QC], psc[:, :, lo:QC], EXP, scale=0.125
                    )
                if j >= 0:
                    nc.vector.tensor_mul(
                        e[:, :, lo:lo + 128], e[:, :, lo:lo + 128], tri_sb[:]
                    )
                if pend is not None:
                    pump(400 if j < 1 else 150)
                    require(f"v{pend[0]}")
                    emit_pv(*pend)
                pend = (ki, e, lo)
            pump(150)
            require(f"v{pend[0]}")
            emit_pv(*pend)
            # free the po banks promptly: numerators+denominator to SBUF via
            # ACT, which is idle exactly at these unit boundaries (no exps);
            # the normalization then runs off the PSUM-release path
            po_sb0 = posbp.tile([65, QC], FP, name="po_sb0", tag="posb")
            po_sb1 = posbp.tile([65, QC], FP, name="po_sb1", tag="posb")
            nc.scalar.copy(po_sb0[:], po0[:])
            nc.scalar.copy(po_sb1[:], po1[:])
            return po_sb0, po_sb1

        def division(pair, qi, po_sb0, po_sb1):
            """att = po / l from the SBUF copies, off the PSUM-release path."""
            qsl = slice(qi * QC, (qi + 1) * QC)
            for sub, posb in enumerate([po_sb0, po_sb1]):
                l_sb = small.tile([1, QC], FP, name="l_sb", tag="lsb", bufs=4)
                nc.vector.tensor_copy(l_sb[:], posb[64:65, :])
                rl = small.tile([1, QC], FP, name="rl", tag="rl", bufs=4)
                nc.vector.reciprocal_approx_fast(out=rl[:], in_=l_sb[:])
                rb_ = small.tile([64, QC], FP, name="rb_", tag="rbb", bufs=4)
                nc.gpsimd.partition_broadcast(rb_[:], rl[:])
                nc.vector.tensor_mul(
                    att_sb[sub * 64:(sub + 1) * 64, pair, qsl], posb[0:64, :], rb_[:]
                )

        # ---- emission schedule ----
        # pre-phase (DMA-bound): just the two units the first attention QKs
        # need, with PSUM->SBUF copies on the otherwise-idle ACT engine;
        # everything else becomes paced filler, flushed on demand by the
        # fine-grained require() calls inside attention_unit.
        for rb, t in [(2, 0), (0, 1)]:
            ps = proj_a(rb, t)
            proj_b(rb, t, ps, act=True)
            emitted.add(f"p{rb}_{t}")
        for tt in range(4):
            v_unit(tt, act=True)
            emitted.add(f"v{tt}")

        push_proj(2, 1)
        for tt in range(4, 8):
            fillers.append((f"v{tt}", 1000, lambda tt=tt: v_unit(tt)))
        push_proj(3, 0); push_proj(1, 1); push_proj(3, 1)        # att(1, p1)
        push_proj(0, 0); push_proj(1, 0)                         # att(0)
        push_proj(0, 2); push_proj(2, 2); push_v(2)              # att(2, p0)
        push_proj(1, 2); push_proj(3, 2)                         # att(2, p1)
        push_proj(0, 3); push_proj(2, 3); push_v(3)              # att(3, p0)
        push_proj(1, 3); push_proj(3, 3)                         # att(3, p1)

        order = [1, 2, 3, 0]
        for qi in order:
            for pair in range(2):
                pos = attention_unit(pair, qi)
                division(pair, qi, *pos)
            if qi != order[-1]:
                push_oproj(qi)

        # drain leftovers, then the final o_proj chunk with copies alternating
        # between the now-idle ACT and DVE, per-ot DMAs over three queues
        require(*[n for n, _, _ in list(fillers)])
        ql = order[-1]
        tail_q = [nc.sync, nc.gpsimd, nc.scalar]
        for half in range(2):
            ow = work.tile([128, 4, QC], BF, name="ow", tag="ow")
            for oi in range(4):
                qsl = slice(ql * QC, (ql + 1) * QC)
                ot = half * 4 + oi
                pw = ps_main.tile([128, QC], FP, name="pw", tag="ps")
                for p in range(2):
                    nc.tensor.matmul(
                        pw[:], wo_sb[:, p, ot * 128:(ot + 1) * 128],
                        att_sb[:, p, qsl], start=(p == 0), stop=(p == 1),
                    )
                if oi % 2 == 0:
                    nc.scalar.copy(ow[:, oi, :], pw[:])
                else:
                    nc.vector.tensor_copy(ow[:, oi, :], pw[:])
                off = (ql * 2 + half) * 4 * QC + oi * QC
                tail_q[(half * 4 + oi) % 3].dma_start(
                    outT[:, off:off + QC], ow[:, oi, :],
                )


# ---------- host-side shard preparation ----------

def make_core_inputs(hidden_states, cos, sin, w_qkv, w_o):
    """Returns list of 8 in_maps (numpy, bf16 where needed)."""
    bf = ml_dtypes.bfloat16
    hs = np.asarray(hidden_states, np.float32)
    cos = np.asarray(cos, np.float32)
    sin = np.asarray(sin, np.float32)
    w_qkv = np.asarray(w_qkv, np.float32)
    w_o = np.asarray(w_o, np.float32)

    cosT = cos.T
    sinT = sin.T
    cos2T = np.ascontiguousarray(cosT).astype(bf)
    ssin2T = np.ascontiguousarray(sinT).astype(bf)
    # signed rotate-half permutation: out[m] = sign(m) * x[partner(m)]
    # lhsT layout: pmat[k, m] = sign(m) at k = partner(m)
    pmat = np.zeros((128, 128), np.float32)
    for m in range(128):
        d = m % 64
        base = m - d
        if d < 32:
            pmat[base + d + 32, m] = -1.0
        else:
            pmat[base + d - 32, m] = 1.0
    pmat = pmat.astype(bf)

    kp = np.arange(128)[:, None]
    cc = np.arange(128)[None, :]
    tri = (kp <= cc).astype(bf)
    maskD = np.concatenate([tri, tri], axis=1)

    def swz(a):
        # [K*128, M] -> [128, K*M] partition-major (matches SBUF tiles)
        k = a.shape[0] // 128
        return np.ascontiguousarray(
            a.reshape(k, 128, a.shape[1]).transpose(1, 0, 2).reshape(128, -1))

    in_maps = []
    for c in range(8):
        b, g = divmod(c, 4)
        heads = range(4 * g, 4 * g + 4)
        hsT = np.ascontiguousarray(hs[b].T).astype(bf)  # [HID, S]
        # chunk-major swizzle: [128, t, kk, s']
        hs2 = np.ascontiguousarray(
            hsT.reshape(8, 128, 4, 512).transpose(1, 2, 0, 3).reshape(128, -1))
        wq = np.concatenate([w_qkv[h * 64:(h + 1) * 64] for h in heads], 0)
        wk = np.concatenate([w_qkv[HID + h * 64:HID + (h + 1) * 64] for h in heads], 0)
        wv = np.concatenate([w_qkv[2 * HID + h * 64:2 * HID + (h + 1) * 64] for h in heads], 0)
        wqkT = swz(np.ascontiguousarray(np.concatenate([wq, wk], 0).T).astype(bf))
        wvT = swz(np.ascontiguousarray(wv.T).astype(bf))
        woT = swz(np.ascontiguousarray(
            np.concatenate([w_o[:, h * 64:(h + 1) * 64] for h in heads], 1).T
        ).astype(bf))
        in_maps.append({
            "hsT": hs2, "wqkT": wqkT, "wvT": wvT, "woT": woT,
            "cos2T": cos2T, "ssin2T": ssin2T, "maskD": maskD, "pmat": pmat,
        })
    return in_maps


def unswizzle_out(o2):
    # [128, qi*half*oi*512] -> outT [1024, 2048]
    a = o2.reshape(128, NQC, 2, 4, QC)
    return np.ascontiguousarray(
        a.transpose(2, 3, 0, 1, 4).reshape(HID, S))


def unshard(outTs):
    out = np.zeros((B, S, HID), np.float32)
    for c, o2 in enumerate(outTs):
        out[c // 4] += unswizzle_out(o2).T.astype(np.float32)
    return out


# ---------- standalone kernel entry ----------

from concourse.bass_utils import run_bass_kernel_spmd

_CACHED_NC = None


def get_program():
    global _CACHED_NC
    if _CACHED_NC is None:
        _CACHED_NC = build_program()
    return _CACHED_NC


def run(inputs, trace=False):
    nc = get_program()
    in_maps = make_core_inputs(**inputs)
    res = run_bass_kernel_spmd(nc, in_maps, core_ids=list(range(8)), trace=trace)
    out = np.zeros((B, S, HID), np.float32)
    for c, r in enumerate(res.results):
        out[c // 4] += unswizzle_out(r["outT"]).T.astype(np.float32)
    return out, res


def kernel(**inputs):
    out, _ = run(inputs, trace=False)
    return out
